# revision 7
# baseline (speedup 1.0000x reference)
"""Trainium2 Bass kernel for CronRootAttention (sparse attention).

Shapes (hardcoded): B=2 H=16 S=4096 D=128, W=64, NB=R=64.
Sharding: fused B*H=32 axis split across 8 cores (4 slices/core).

Per-core device kernel (per (b,h) slice, per 128-query tile i):
  scores[j, n] layout: [local 192 keys | strided 64 | relay 64] (tile 0: [local 128 | sr 128])
  - QK matmuls: lhsT = qT tile (d-major, host-pretransposed bf16), rhs = kT column windows
  - mask add via identity-stationary matmuls streaming bf16 mask constants into PSUM
  - ACT exp (scale=1/sqrt(D)) PSUM->SBUF bf16 p, accum_out = row sums (fp32)
  - PE transposes of p chunks -> pT (bf16 PSUM), one DVE copyout -> SBUF
  - PV matmuls: lhsT = pT chunks, rhs = v row-tiles (64-shifted so windows are single APs)
  - DVE reciprocal + per-partition scale, DMA out (fp32)
"""

import numpy as np
import ml_dtypes

import concourse.bass as bass
import concourse.bacc as bacc
import concourse.tile as tile
from concourse import mybir
from concourse.bass_utils import run_bass_kernel_spmd

BF16 = ml_dtypes.bfloat16
B, H, S, D = 2, 16, 4096, 128
W = 64
NB = S // W          # 64
R = NB               # 64
NCORES = 8
SLICES = B * H // NCORES   # 4 (b,h) slices per core
NT = S // 128        # 32 query tiles per slice
NEG = np.float32(-1e30)
SCALE = 1.0 / np.sqrt(np.float32(D))

_prog_cache = {}


def _build_masks():
    j = np.arange(128)[:, None]
    # local mask, tiles i>=1: key span = 3 blocks [128i-64, 128i+128), window cols [j+1, j+64]
    c = np.arange(192)[None, :]
    mloc = np.where((c >= j + 1) & (c <= j + 64), 0.0, NEG).astype(np.float32)
    # local mask, tile 0: key span = [0, 128), window cols [max(j-63,0), j]
    c0 = np.arange(128)[None, :]
    mloc0 = np.where((c0 >= np.maximum(j - 63, 0)) & (c0 <= j), 0.0, NEG).astype(np.float32)
    # sliding strided/relay masks [128, 256] = [M_str | M_rel], tile i uses cols
    # [64-2i : 128-2i] of each half (c-64 == s-2i resp. r-2i).
    cc = np.arange(128)[None, :]
    p = np.arange(128)[:, None]
    mstr = np.where(cc < 64 + (p >= 64).astype(np.int64), 0.0, NEG).astype(np.float32)
    mrel = np.where(p >= 64 * (cc - 64) + 127, 0.0, NEG).astype(np.float32)
    msr = np.concatenate([mstr, mrel], axis=1)
    return mloc.astype(BF16), mloc0.astype(BF16), msr.astype(BF16)


def build_program():
    if "nc" in _prog_cache:
        return _prog_cache["nc"]
    dt = mybir.dt
    nc = bacc.Bacc("TRN2", target_bir_lowering=False, debug=False)

    qT_d = nc.declare_dram_parameter("qT", [SLICES, 128, S], dt.bfloat16, isOutput=False)
    kT_d = nc.declare_dram_parameter("kT", [SLICES, 128, S], dt.bfloat16, isOutput=False)
    vsh_d = nc.declare_dram_parameter("vsh", [SLICES, 128, 33 * 128], dt.bfloat16, isOutput=False)
    kTsr_d = nc.declare_dram_parameter("kTsr", [SLICES, 128, 128], dt.bfloat16, isOutput=False)
    vsr_d = nc.declare_dram_parameter("vsr", [SLICES, 128, 128], dt.bfloat16, isOutput=False)
    vn0_d = nc.declare_dram_parameter("vn0", [SLICES, 128, 128], dt.bfloat16, isOutput=False)
    ident_d = nc.declare_dram_parameter("ident", [128, 128], dt.bfloat16, isOutput=False)
    mloc_d = nc.declare_dram_parameter("mloc", [128, 192], dt.bfloat16, isOutput=False)
    mloc0_d = nc.declare_dram_parameter("mloc0", [128, 128], dt.bfloat16, isOutput=False)
    msr_d = nc.declare_dram_parameter("msr", [128, 256], dt.bfloat16, isOutput=False)
    out_d = nc.declare_dram_parameter("out", [SLICES, S, D], dt.float32, isOutput=True)

    from contextlib import ExitStack
    with tile.TileContext(nc) as tc, ExitStack() as ctx:
        cpool = ctx.enter_context(tc.tile_pool(name="consts", bufs=1))
        ident = cpool.tile([128, 128], dt.bfloat16, tag="ident")
        nc.sync.dma_start(ident[:], ident_d[:, :])
        mloc = cpool.tile([128, 192], dt.bfloat16, tag="mloc")
        nc.sync.dma_start(mloc[:], mloc_d[:, :])
        mloc0 = cpool.tile([128, 128], dt.bfloat16, tag="mloc0")
        nc.sync.dma_start(mloc0[:], mloc0_d[:, :])
        msr = cpool.tile([128, 256], dt.bfloat16, tag="msr")
        nc.sync.dma_start(msr[:], msr_d[:, :])

        spool = ctx.enter_context(tc.tile_pool(name="slice_in", bufs=2))
        pscores = ctx.enter_context(tc.tile_pool(name="pscores", bufs=2, space="PSUM"))
        ppt = ctx.enter_context(tc.tile_pool(name="ppt", bufs=2, space="PSUM"))
        pout = ctx.enter_context(tc.tile_pool(name="pout", bufs=2, space="PSUM"))
        wpool = ctx.enter_context(tc.tile_pool(name="work", bufs=3))

        for s in range(SLICES):
            qT = spool.tile([128, S], dt.bfloat16, tag="qT")
            nc.sync.dma_start(qT[:], qT_d[s])
            kT = spool.tile([128, S], dt.bfloat16, tag="kT")
            nc.sync.dma_start(kT[:], kT_d[s])
            vsh = spool.tile([128, 33 * 128], dt.bfloat16, tag="vsh")
            nc.sync.dma_start(vsh[:], vsh_d[s])
            kTsr = spool.tile([128, 128], dt.bfloat16, tag="kTsr")
            nc.sync.dma_start(kTsr[:], kTsr_d[s])
            vsr = spool.tile([128, 128], dt.bfloat16, tag="vsr")
            nc.sync.dma_start(vsr[:], vsr_d[s])
            vn0 = spool.tile([128, 128], dt.bfloat16, tag="vn0")
            nc.sync.dma_start(vn0[:], vn0_d[s])

            for i in range(NT):
                lw = 128 if i == 0 else 192       # local key-span width
                tw = lw + 128                     # total scores width
                scores = pscores.tile([128, 320], dt.float32, tag="scores")
                qTi = qT[:, 128 * i:128 * (i + 1)]
                # QK: local window then strided+relay, q stationary
                if i == 0:
                    kloc = kT[:, 0:128]
                else:
                    kloc = kT[:, 128 * i - 64:128 * i + 128]
                # NOTE: keep each column-range's accumulation group sequential
                # (start..stop before the next range's start) — an interleaved
                # start=True clears the whole PSUM bank, wiping the open range.
                mloc_i = mloc0[:, :] if i == 0 else mloc[:, :]
                nc.tensor.matmul(scores[:, 0:lw], qTi, kloc,
                                 start=True, stop=False)
                nc.tensor.matmul(scores[:, 0:lw], ident[:], mloc_i,
                                 start=False, stop=True)
                nc.tensor.matmul(scores[:, lw:tw], qTi, kTsr[:, :],
                                 start=True, stop=False, skip_group_check=True)
                msr_i = msr[:].rearrange("p (g c) -> p g c", g=2)[:, :, 64 - 2 * i:128 - 2 * i]
                nc.tensor.matmul(scores[:, lw:tw], ident[:], msr_i,
                                 start=False, stop=True, skip_group_check=True)

                # softmax numerator + row sums
                p_sb = wpool.tile([128, 320], dt.bfloat16, tag="p_sb")
                sums = wpool.tile([128, 1], dt.float32, tag="sums")
                nc.scalar.activation(p_sb[:, 0:tw], scores[:, 0:tw],
                                     mybir.ActivationFunctionType.Exp,
                                     scale=float(SCALE), accum_out=sums[:])

                # transpose p chunks -> pT (bf16, PSUM), single copyout
                ptp = ppt.tile([128, 384], dt.bfloat16, tag="ptp")
                nc.tensor.transpose(ptp[:, 0:128], p_sb[:, 0:128], ident[:])
                if i == 0:
                    nc.tensor.transpose(ptp[:, 128:256], p_sb[:, 128:256], ident[:])
                    cw = 256
                else:
                    nc.tensor.transpose(ptp[0:64, 128:256], p_sb[:, 128:192], ident[:])
                    nc.tensor.transpose(ptp[:, 256:384], p_sb[:, 192:320], ident[:])
                    cw = 384
                pt = wpool.tile([128, 384], dt.bfloat16, tag="pt")
                nc.vector.tensor_copy(pt[:, 0:cw], ptp[:, 0:cw])

                # PV
                outp = pout.tile([128, 128], dt.float32, tag="outp")
                if i == 0:
                    nc.tensor.matmul(outp[:], pt[:, 0:128], vn0[:],
                                     start=True, stop=False)
                    nc.tensor.matmul(outp[:], pt[:, 128:256], vsr[:],
                                     start=False, stop=True)
                else:
                    nc.tensor.matmul(outp[:], pt[:, 0:128],
                                     vsh[:, 128 * i:128 * (i + 1)],
                                     start=True, stop=False)
                    nc.tensor.matmul(outp[:], pt[0:64, 128:256],
                                     vsh[0:64, 128 * (i + 1):128 * (i + 2)],
                                     start=False, stop=False)
                    nc.tensor.matmul(outp[:], pt[:, 256:384], vsr[:],
                                     start=False, stop=True)

                # normalize + store
                rsum = wpool.tile([128, 1], dt.float32, tag="rsum")
                nc.vector.reciprocal(rsum[:], sums[:])
                out_sb = wpool.tile([128, 128], dt.float32, tag="out_sb")
                nc.vector.tensor_scalar_mul(out_sb[:], outp[:], rsum[:])
                nc.sync.dma_start(out_d[s, 128 * i:128 * (i + 1), :], out_sb[:])

    nc.finalize()
    _prog_cache["nc"] = nc
    return nc


def _prep_core_inputs(q, k, v, rk, rv, masks, ident):
    """q,k,v: [SLICES, S, D] fp32 for one core; rk, rv: [SLICES, R, D]."""
    mloc, mloc0, msr = masks
    qb = q.astype(BF16)
    kb = k.astype(BF16)
    vb = v.astype(BF16)
    qT = np.ascontiguousarray(qb.transpose(0, 2, 1))          # [SL, 128, S]
    kT = np.ascontiguousarray(kb.transpose(0, 2, 1))
    # 64-shifted padded v: vpad = [zeros(64); v; zeros(64)], tiles of 128 rows,
    # stored per-partition-contiguous: [SL, 128, 33*128]
    vpad = np.concatenate([np.zeros((SLICES, 64, D), BF16), vb,
                           np.zeros((SLICES, 64, D), BF16)], axis=1)  # [SL, 4224, D]
    vsh = np.ascontiguousarray(
        vpad.reshape(SLICES, 33, 128, D).transpose(0, 2, 1, 3).reshape(SLICES, 128, 33 * 128))
    ksr = np.concatenate([kb[:, ::W, :], rk.astype(BF16)], axis=1)    # [SL, 128, D]
    kTsr = np.ascontiguousarray(ksr.transpose(0, 2, 1))               # [SL, 128, 128]
    vsr = np.ascontiguousarray(np.concatenate([vb[:, ::W, :], rv.astype(BF16)], axis=1))
    vn0 = np.ascontiguousarray(vb[:, 0:128, :])
    return {
        "qT": qT, "kT": kT, "vsh": vsh, "kTsr": kTsr, "vsr": vsr, "vn0": vn0,
        "ident": ident, "mloc": mloc, "mloc0": mloc0, "msr": msr,
    }


def make_in_maps(q, k, v, rk, rv):
    masks = _build_masks()
    ident = np.eye(128, dtype=BF16)
    qf = q.reshape(B * H, S, D)
    kf = k.reshape(B * H, S, D)
    vf = v.reshape(B * H, S, D)
    rkf = rk.reshape(B * H, R, D)
    rvf = rv.reshape(B * H, R, D)
    in_maps = []
    for c in range(NCORES):
        sl = slice(SLICES * c, SLICES * (c + 1))
        in_maps.append(_prep_core_inputs(qf[sl], kf[sl], vf[sl], rkf[sl], rvf[sl],
                                         masks, ident))
    return in_maps


def kernel(q, k, v, rk, rv, _run_kwargs=None):
    q = np.asarray(q, dtype=np.float32)
    k = np.asarray(k, dtype=np.float32)
    v = np.asarray(v, dtype=np.float32)
    rk = np.asarray(rk, dtype=np.float32)
    rv = np.asarray(rv, dtype=np.float32)
    nc = build_program()
    in_maps = make_in_maps(q, k, v, rk, rv)
    res = run_bass_kernel_spmd(nc, in_maps, list(range(NCORES)), **(_run_kwargs or {}))
    out = np.stack([res.results[c]["out"] for c in range(NCORES)])  # [8, SL, S, D]
    if _run_kwargs:
        kernel.last_results = res
    return out.reshape(B, H, S, D)
